# revision 16
# baseline (speedup 1.0000x reference)
"""Causal self-attention Trainium2 kernel (Bass/Tile), 8-core SPMD.

Problem: X[2, 2048, 1024], W_qkv[1024, 3072], W_proj[1024, 1024], H=16 heads.

Sharding: core c handles batch b = c // 4 and heads h0 = 4*(c % 4) .. h0+4
(tensor-parallel over heads + data-parallel over batch). Each core computes
a partial output  out_b = Y[:, heads] @ W_proj[head rows, :]  and the host
sums the 4 partials per batch (the W_proj row-shard reduction).

Per-core device layout ("transposed attention", no P transposes needed):
  Xt  [C, T]      X[b].T, streamed in [128, 512] tiles
  Qt,Kt [128,2,T] per head-pair group g: partition = 64*(h%2)+d, free = token
  V   [128,16,260] natural [token-block, head*65(+ones col)] for PV lhsT
  St  = Kt_blk.T @ Qt_chunk   -> [keys 128, q 512] PSUM  (K=d=64 contraction,
        both heads of a pair packed into PE row groups 0/64 concurrently)
  P   = exp(0.125*(St + causal_mask))  via ACT, ones-augmented PV gives
  Yt_aug = [V|1].T @ P -> [65, q 512]: rows 0-63 = Yt, row 64 = softmax sums
  yt2 [128, 2, T] normalized Yt stacked per head pair (odd head rows moved
        to partitions 64-127 by an SBUF->SBUF DMA) so the output projection
        runs K=128 matmuls.
"""

import numpy as np

B, T, C, H = 2, 2048, 1024, 16
HD = 64          # head dim
HPC = 4          # heads per core
P = 128
NCORES = 8
CH = 512         # token chunk (matmul free dim / q chunk)
KB = 128         # key block
MASK_VAL = -1.0e5
MM_DTYPE = "bf16"


def build_nc(t_len=T, mm_dtype=None):
    import concourse.bass as bass
    import concourse.mybir as mybir
    from concourse import bacc, library_config
    from concourse.tile import TileContext
    from contextlib import ExitStack

    mm_dtype = mm_dtype or MM_DTYPE
    f32 = mybir.dt.float32
    mdt = mybir.dt.bfloat16 if mm_dtype == "bf16" else mybir.dt.float32r
    Exp = mybir.ActivationFunctionType.Exp
    Alu = mybir.AluOpType

    NKC = C // P          # 8 contraction chunks over C
    NCH = t_len // CH     # token chunks
    NTB = t_len // P      # token blocks
    SC = 1.0 / np.sqrt(HD)

    nc = bacc.Bacc("TRN2", target_bir_lowering=False, debug=False,
                   num_devices=NCORES)

    xt_d = nc.dram_tensor("xt", [C, t_len], mdt, kind="ExternalInput").ap()
    wq_d = nc.dram_tensor("wq", [P, NKC, HPC * HD], mdt, kind="ExternalInput").ap()
    wk_d = nc.dram_tensor("wk", [P, NKC, HPC * HD], mdt, kind="ExternalInput").ap()
    wv_d = nc.dram_tensor("wv", [P, NKC, HPC * HD], mdt, kind="ExternalInput").ap()
    wp_d = nc.dram_tensor("wp", [P, 2, C], mdt, kind="ExternalInput").ap()
    out_d = nc.dram_tensor("out", [t_len, C], f32, kind="ExternalOutput").ap()

    with TileContext(nc) as tc, ExitStack() as ctx:
        nc.gpsimd.load_library(library_config.proxy)
        const = ctx.enter_context(tc.tile_pool(name="const", bufs=1))
        work = ctx.enter_context(tc.tile_pool(name="work", bufs=3))
        xtp = ctx.enter_context(tc.tile_pool(name="xtp", bufs=10))
        psm = ctx.enter_context(tc.tile_pool(name="psm", bufs=2, space="PSUM"))

        # ---- persistent SBUF tensors ----
        wq_sb = const.tile([P, NKC, HPC * HD], mdt, tag="wq")
        wk_sb = const.tile([P, NKC, HPC * HD], mdt, tag="wk")
        wv_sb = const.tile([P, NKC, HPC * HD], mdt, tag="wv")
        wp_sb = const.tile([P, 2, C], mdt, tag="wp")
        qt = const.tile([P, 2, t_len], mdt, tag="qt")
        kt = const.tile([P, 2, t_len], mdt, tag="kt")
        va = const.tile([P, NTB, HPC * (HD + 1)], mdt, tag="va")
        yt2 = const.tile([P, 2, t_len], mdt, tag="yt2")
        masks = const.tile([P, 4, CH], f32, tag="masks")

        nc.sync.dma_start(wq_sb[:], wq_d[:])
        nc.sync.dma_start(wk_sb[:], wk_d[:])
        nc.sync.dma_start(wv_sb[:], wv_d[:])
        nc.sync.dma_start(wp_sb[:], wp_d[:])

        # prefetch the first token chunk of Xt before generating masks
        xts0 = []
        for kc in range(NKC):
            xt_t = xtp.tile([P, CH], mdt, tag="xt", name="xt_t0")
            nc.sync.dma_start(xt_t[:], xt_d[kc * P:(kc + 1) * P, 0:CH])
            xts0.append(xt_t)

        # causal masks for the 4 diagonal offsets: keep (0) where
        # q_local >= k_local + o, else MASK_VAL.  iota = -k + q - o >= 0
        nc.vector.memset(masks[:], 0.0)
        for o4 in range(4):
            nc.gpsimd.affine_select(
                out=masks[:, o4, :], in_=masks[:, o4, :],
                compare_op=Alu.is_ge, fill=MASK_VAL,
                base=-(o4 * KB), channel_multiplier=-1, pattern=[[1, CH]],
            )
        # ones columns of the augmented V (softmax denominator trick)
        for hl in range(HPC):
            nc.vector.memset(va[:, :, hl * (HD + 1) + HD: hl * (HD + 1) + HD + 1], 1.0)

        # ---- phase 1: QKV projections ----
        for tch in range(NCH):
            if tch == 0:
                xts = xts0
            else:
                xts = []
                for kc in range(NKC):
                    xt_t = xtp.tile([P, CH], mdt, tag="xt")
                    nc.sync.dma_start(
                        xt_t[:],
                        xt_d[kc * P:(kc + 1) * P, tch * CH:(tch + 1) * CH])
                    xts.append(xt_t)
            for w_sb, dst in ((wq_sb, qt), (wk_sb, kt)):
                for g in range(2):
                    pq = psm.tile([P, 2 * CH], f32, tag="st", bufs=3,
                                  name="pq")[:, :CH]
                    for kc in range(NKC):
                        nc.tensor.matmul(
                            pq[:],
                            lhsT=w_sb[:, kc, g * P:(g + 1) * P],
                            rhs=xts[kc][:],
                            start=(kc == 0), stop=(kc == NKC - 1))
                    nc.vector.tensor_copy(
                        out=dst[:, g, tch * CH:(tch + 1) * CH], in_=pq[:])
            for vb in range(CH // P):
                tb = tch * (CH // P) + vb
                pv = psm.tile([P, 2 * CH], f32, tag="st", bufs=3,
                              name="pv")[:, :HPC * HD]
                for kc in range(NKC):
                    nc.tensor.matmul(
                        pv,
                        lhsT=xts[kc][:, vb * P:(vb + 1) * P],
                        rhs=wv_sb[:, kc, :],
                        start=(kc == 0), stop=(kc == NKC - 1))
                nc.vector.tensor_copy(
                    out=va[:, tb, :].rearrange("p (h e) -> p h e", e=HD + 1)[:, :, :HD],
                    in_=pv.rearrange("p (h e) -> p h e", e=HD))

        # ---- phase 2: attention (causal, St layout [keys, q]) ----
        def normalize_and_store(g, hh, qi, ytp):
            """Divide Yt rows by the sums row and store into yt2."""
            qsl = slice(qi * CH, (qi + 1) * CH)
            bnc = work.tile([HD + 1, CH], f32, tag="bounce", bufs=4)
            nc.vector.tensor_copy(out=bnc[HD:HD + 1, :], in_=ytp[HD:HD + 1, :])
            # partition_broadcast / custom-DVE ops ignore the AP base
            # partition on HW: DMA the sums row to partition 0, broadcast,
            # then reciprocal on the base-0 tile.
            s0 = work.tile([1, CH], f32, tag="s0", bufs=4)
            nc.sync.dma_start(s0[0:1, :], bnc[HD:HD + 1, :])
            rb = work.tile([HD, CH], f32, tag="rb", bufs=4)
            nc.gpsimd.partition_broadcast(rb[:], s0[0:1, :])
            nc.vector.reciprocal_approx_fast(out=rb[:], in_=rb[:])
            if hh == 0:
                nc.vector.tensor_tensor(out=yt2[0:HD, g, qsl],
                                        in0=ytp[:HD, :], in1=rb[:],
                                        op=Alu.mult)
            else:
                ybs = work.tile([HD, CH], mdt, tag="ybs", bufs=3)
                nc.vector.tensor_tensor(out=ybs[:], in0=ytp[:HD, :],
                                        in1=rb[:], op=Alu.mult)
                # move the odd head's rows to partitions 64-127
                nc.sync.dma_start(yt2[HD:P, g, qsl], ybs[:])

        for qi in range(NCH):
            nkb = (qi + 1) * (CH // KB)
            for g in range(2):
                ytps = [psm.tile([HD + 1, CH], f32, tag="yt", bufs=2,
                                 name=f"ytp{hh}") for hh in range(2)]
                for kb2 in range(nkb // 2):
                    stps = [psm.tile([P, 2 * CH], f32, tag="st", bufs=3,
                                     name=f"stp{hh}") for hh in range(2)]
                    pts = [work.tile([P, 2 * CH], mdt, tag="p", bufs=4,
                                     name=f"pt{hh}") for hh in range(2)]
                    for j in range(2):
                        kb = kb2 * 2 + j
                        for hh in range(2):     # packed PE row groups 0 / 64
                            nc.tensor.matmul(
                                stps[hh][:, j * CH:(j + 1) * CH],
                                lhsT=kt[hh * HD:(hh + 1) * HD, g,
                                        kb * KB:(kb + 1) * KB],
                                rhs=qt[hh * HD:(hh + 1) * HD, g,
                                       qi * CH:(qi + 1) * CH],
                                start=True, stop=True)
                        o4 = kb - (CH // KB) * qi
                        if o4 >= 0:    # diagonal block: apply causal mask
                            wm = (o4 + 1) * KB
                            for hh in range(2):
                                nc.vector.tensor_tensor(
                                    out=stps[hh][:, j * CH:j * CH + wm],
                                    in0=stps[hh][:, j * CH:j * CH + wm],
                                    in1=masks[:, o4, :wm], op=Alu.add)
                    for hh in range(2):
                        nc.scalar.activation(out=pts[hh][:], in_=stps[hh][:],
                                             func=Exp, scale=SC)
                    for hh in range(2):
                        hl = 2 * g + hh
                        for j in range(2):
                            kb = kb2 * 2 + j
                            nc.tensor.matmul(
                                ytps[hh][:],
                                lhsT=va[:, kb, hl * (HD + 1):(hl + 1) * (HD + 1)],
                                rhs=pts[hh][:, j * CH:(j + 1) * CH],
                                start=(kb == 0), stop=(kb == nkb - 1))
                for hh in range(2):
                    normalize_and_store(g, hh, qi, ytps[hh])

            # ---- phase 3: output projection for this qi's tokens ----
            for tb in range(qi * (CH // P), (qi + 1) * (CH // P)):
                for oc in range(C // CH):
                    pp = psm.tile([P, 2 * CH], f32, tag="st", bufs=3,
                                  name="pp")[:, :CH]
                    for yc in range(2):
                        nc.tensor.matmul(
                            pp[:],
                            lhsT=yt2[:, yc, tb * P:(tb + 1) * P],
                            rhs=wp_sb[:, yc, oc * CH:(oc + 1) * CH],
                            start=(yc == 0), stop=(yc == 1))
                    ost = work.tile([P, CH], f32, tag="ost", bufs=3)
                    nc.vector.tensor_copy(out=ost[:], in_=pp[:])
                    nc.sync.dma_start(
                        out_d[tb * P:(tb + 1) * P, oc * CH:(oc + 1) * CH],
                        ost[:])
    nc.compile()
    return nc


def _to_mm_dtype(a):
    if MM_DTYPE == "bf16":
        import ml_dtypes
        return np.ascontiguousarray(a).astype(ml_dtypes.bfloat16)
    return np.ascontiguousarray(a).astype(np.float32)


def make_in_maps(X, W_qkv, W_proj, t_len=T):
    """Host-side sharding: slice + pre-arrange weights per head group,
    transpose X.  Layouts match the SBUF tensors so every weight DMA is
    fully contiguous:
      wq/wk/wv [128, 8, 256]: [p, kc, m] = W[kc*128+p, cols][m]
      wp       [128, 2, C]:   [64*hh+d, yc, m] = W_proj[(2*yc+hh)*64+d, m]
    """
    in_maps = []
    xts = [_to_mm_dtype(np.asarray(X[b, :t_len, :]).T) for b in range(B)]
    NKC = C // P
    for c in range(NCORES):
        b = c // (NCORES // B)
        h0 = HPC * (c % (NCORES // B))
        cols = slice(h0 * HD, (h0 + HPC) * HD)

        def warr(w):
            return _to_mm_dtype(
                np.ascontiguousarray(w).reshape(NKC, P, HPC * HD)
                .transpose(1, 0, 2))

        wp_c = np.ascontiguousarray(W_proj[cols, :])          # [256, C]
        wp2 = wp_c.reshape(2, 2, HD, C).transpose(1, 2, 0, 3).reshape(P, 2, C)
        in_maps.append({
            "xt": xts[b],
            "wq": warr(W_qkv[:, cols]),
            "wk": warr(W_qkv[:, C:][:, cols]),
            "wv": warr(W_qkv[:, 2 * C:][:, cols]),
            "wp": _to_mm_dtype(wp2),
        })
    return in_maps


_CACHE = {}
TRACE = False           # set True (e.g. from test.py) to capture an NTFF profile


def kernel(X, W_qkv, W_proj):
    import sys
    if "/opt/trn_rl_repo" not in sys.path:
        sys.path.insert(0, "/opt/trn_rl_repo")
    from concourse.bass_utils import run_bass_kernel_spmd

    X = np.asarray(X, dtype=np.float32)
    W_qkv = np.asarray(W_qkv, dtype=np.float32)
    W_proj = np.asarray(W_proj, dtype=np.float32)

    if "nc" not in _CACHE:
        _CACHE["nc"] = build_nc()
    nc = _CACHE["nc"]

    in_maps = make_in_maps(X, W_qkv, W_proj)
    res = run_bass_kernel_spmd(nc, in_maps, core_ids=list(range(NCORES)),
                               trace=TRACE)
    _CACHE["last"] = res
    out = np.empty((B, T, C), dtype=np.float32)
    ncb = NCORES // B
    for b in range(B):
        acc = res.results[b * ncb]["out"].astype(np.float32)
        for c in range(b * ncb + 1, (b + 1) * ncb):
            acc = acc + res.results[c]["out"]
        out[b] = acc
    return out
